# revision 6
# baseline (speedup 1.0000x reference)
"""InputScaledQuantLinear on 8 TRN2 NeuronCores.

out = dq(fp8_quant(x / s)) * s @ W^T + bias

Sharding: x rows split 8 ways (data parallel), weight/bias replicated.
Host pre-packs per-shard inputs so every device DMA is contiguous:
  - xqT: e4m3(x/s) transposed to [IN, NS] (the e4m3 quantization is
    bit-identical to the reference, so that error cancels)
  - wT:  (s * W)^T as [IN, OUT] bf16
Device is then a pure dense bf16-rate GEMM: stationary = xq^T row-tile,
moving = wT out-slice, accumulate K=2048 in PSUM, bias-add on DVE.
"""

import numpy as np
import ml_dtypes
from contextlib import ExitStack

import concourse.bass as bass
import concourse.mybir as mybir
import concourse.tile as tile
from concourse import bacc
from concourse.bass_utils import run_bass_kernel_spmd

N_CORES = 8
N, IN, OUT = 32768, 2048, 2048
NS = N // N_CORES          # 4096 rows per core
N_CHUNK = 1024             # token rows per outer iteration
K_TILES = IN // 128        # 16
O_BANKS = OUT // 512       # 4
RT = N_CHUNK // 128        # 8 row-tiles per chunk

_cache = {}


def build():
    nc = bacc.Bacc(trn_type="TRN2")
    xqT = nc.dram_tensor("xqT", [IN, NS], mybir.dt.float8e4, kind="ExternalInput")
    wT = nc.dram_tensor("wT", [IN, OUT], mybir.dt.bfloat16, kind="ExternalInput")
    b = nc.dram_tensor("bias", [OUT], mybir.dt.bfloat16, kind="ExternalInput")
    out = nc.dram_tensor("out", [NS, OUT], mybir.dt.bfloat16, kind="ExternalOutput")

    with tile.TileContext(nc) as tc, ExitStack() as ctx:
        consts = ctx.enter_context(tc.tile_pool(name="consts", bufs=1))
        xp = ctx.enter_context(tc.tile_pool(name="xp", bufs=4))
        op = ctx.enter_context(tc.tile_pool(name="op", bufs=12))
        psum = ctx.enter_context(tc.tile_pool(name="psum", bufs=2, space="PSUM"))

        # ---- x chunk loads: 16 contiguous [128, N_CHUNK] fp8 tiles ----
        def load_chunk(c):
            tiles = []
            for k in range(K_TILES):
                xt = xp.tile([128, N_CHUNK], mybir.dt.float8e4, name=f"xq{k}",
                             tag=f"xq{k}")
                nc.sync.dma_start(
                    xt[:], xqT[k * 128:(k + 1) * 128,
                               c * N_CHUNK:(c + 1) * N_CHUNK])
                tiles.append(xt)
            return tiles

        # ---- bias row (tiny, lands first) ----
        bias_row = consts.tile([1, OUT], mybir.dt.bfloat16)
        nc.scalar.dma_start(bias_row[:], b.rearrange("(p o) -> p o", p=1))
        ones_col = consts.tile([1, 128], mybir.dt.bfloat16)
        nc.vector.memset(ones_col[:], 1.0)

        # ---- startup-critical loads split across BOTH hwdge queues so
        # descriptor issue (~0.6us per DMA instruction) parallelizes:
        # sync queue carries chunk-0 x, scalar queue carries W out-slice 0.
        # The remaining W slices follow on the sync queue; they are needed
        # ~27us/slice later, well after their issue+stream time. ----
        wt_tiles = [consts.tile([128, OUT], mybir.dt.bfloat16, name=f"wt{k}")
                    for k in range(K_TILES)]
        xq0 = load_chunk(0)
        for k in range(K_TILES):
            nc.scalar.dma_start(wt_tiles[k][:, 0:512],
                                wT[k * 128:(k + 1) * 128, 0:512])
        for ob in range(1, O_BANKS):
            for k in range(K_TILES):
                nc.sync.dma_start(
                    wt_tiles[k][:, ob * 512:(ob + 1) * 512],
                    wT[k * 128:(k + 1) * 128, ob * 512:(ob + 1) * 512])

        # ---- bias broadcast via ones-matmul; repeated rounds double as
        # PE warmup so the HAM clock gate opens before the real GEMM ----
        bias_bc = consts.tile([128, OUT], mybir.dt.float32)
        for rep in range(5):
            for ob in range(O_BANKS):
                pt = psum.tile([128, 512], mybir.dt.float32, name="pt",
                               tag=f"acc{ob}")
                nc.tensor.matmul(pt[:], ones_col[:],
                                 bias_row[:, ob * 512:(ob + 1) * 512])
                if rep == 4:
                    nc.scalar.copy(bias_bc[:, ob * 512:(ob + 1) * 512], pt[:])

        # ---- main loop: ob-outer so only one W out-slice gates startup ----
        for c in range(NS // N_CHUNK):
            xqs = xq0 if c == 0 else load_chunk(c)
            n0 = c * N_CHUNK
            for ob in range(O_BANKS):
                o0 = ob * 512
                for rt in range(RT):
                    ps = psum.tile([128, 512], mybir.dt.float32,
                                   name=f"acc{rt % 4}", tag=f"acc{rt % 4}")
                    for k in range(K_TILES):
                        nc.tensor.matmul(
                            ps[:],
                            xqs[k][:, rt * 128:(rt + 1) * 128],
                            wt_tiles[k][:, o0:o0 + 512],
                            start=(k == 0), stop=(k == K_TILES - 1))
                    ot = op.tile([128, 512], mybir.dt.bfloat16, name="ot")
                    nc.vector.tensor_add(ot[:], ps[:], bias_bc[:, o0:o0 + 512])
                    nc.scalar.dma_start(
                        out[n0 + rt * 128:n0 + (rt + 1) * 128, o0:o0 + 512],
                        ot[:])
    nc.finalize()
    return nc


def _quantize_host(x, scale):
    # Bit-identical to reference.quantize_dequant_fp8's quantization step:
    # bf16 divide by bf16 scale, then RNE cast to float8_e4m3fn.
    xb = x.astype(ml_dtypes.bfloat16)
    if scale != 1.0:
        xb = (xb / np.array(scale, dtype=ml_dtypes.bfloat16)).astype(
            ml_dtypes.bfloat16)
    return xb.astype(ml_dtypes.float8_e4m3fn)


def kernel(x, weight, bias, input_scale, _trace=False):
    s = float(np.asarray(input_scale).reshape(-1)[0])
    if "nc" not in _cache:
        _cache["nc"] = build()
    nc = _cache["nc"]

    if s != 1.0:
        wT = np.ascontiguousarray(
            (weight.astype(np.float32) * s).astype(ml_dtypes.bfloat16).T)
    else:
        wT = np.ascontiguousarray(np.asarray(weight).T)
    bias = np.ascontiguousarray(bias)
    xq = _quantize_host(np.asarray(x), s)          # [N, IN] fp8
    in_maps = [
        {"xqT": np.ascontiguousarray(xq[i * NS:(i + 1) * NS].T),
         "wT": wT, "bias": bias}
        for i in range(N_CORES)
    ]
    res = run_bass_kernel_spmd(nc, in_maps, core_ids=list(range(N_CORES)),
                               trace=_trace)
    outs = [res.results[i]["out"] for i in range(N_CORES)]
    full = np.concatenate(outs, axis=0)
    if _trace:
        return full, res
    return full


# revision 9
# speedup vs baseline: 1.0050x; 1.0050x over previous
"""InputScaledQuantLinear on 8 TRN2 NeuronCores.

out = dq(fp8_quant(x / s)) * s @ W^T + bias

Sharding: x rows split 8 ways (data parallel), weight/bias replicated.
Host pre-packs per-shard inputs so every device DMA is contiguous:
  - xqT: e4m3(x/s) transposed to [IN, NS] (the e4m3 quantization is
    bit-identical to the reference, so that error cancels)
  - wT:  (s * W)^T as [IN, OUT] bf16
Device is then a pure dense bf16-rate GEMM: stationary = xq^T row-tile,
moving = wT out-slice, accumulate K=2048 in PSUM, bias-add on DVE.
"""

import numpy as np
import ml_dtypes
from contextlib import ExitStack

import concourse.bass as bass
import concourse.mybir as mybir
import concourse.tile as tile
from concourse import bacc
from concourse.bass_utils import run_bass_kernel_spmd

N_CORES = 8
N, IN, OUT = 32768, 2048, 2048
NS = N // N_CORES          # 4096 rows per core
N_CHUNK = 1024             # token rows per outer iteration
K_TILES = IN // 128        # 16
O_BANKS = OUT // 512       # 4
RT = N_CHUNK // 128        # 8 row-tiles per chunk

_cache = {}


def build():
    nc = bacc.Bacc(trn_type="TRN2")
    xqT = nc.dram_tensor("xqT", [IN, NS], mybir.dt.float8e4, kind="ExternalInput")
    wT = nc.dram_tensor("wT", [IN, OUT], mybir.dt.bfloat16, kind="ExternalInput")
    b = nc.dram_tensor("bias", [OUT], mybir.dt.bfloat16, kind="ExternalInput")
    out = nc.dram_tensor("out", [NS, OUT], mybir.dt.bfloat16, kind="ExternalOutput")

    with tile.TileContext(nc) as tc, ExitStack() as ctx:
        consts = ctx.enter_context(tc.tile_pool(name="consts", bufs=1))
        xp = ctx.enter_context(tc.tile_pool(name="xp", bufs=2))
        op = ctx.enter_context(tc.tile_pool(name="op", bufs=8))
        psum = ctx.enter_context(tc.tile_pool(name="psum", bufs=2, space="PSUM"))

        # ---- x chunk loads: 16 contiguous [128, N_CHUNK] fp8 tiles ----
        def load_chunk(c):
            tiles = []
            for k in range(K_TILES):
                xt = xp.tile([128, N_CHUNK], mybir.dt.float8e4, name=f"xq{k}",
                             tag=f"xq{k}")
                nc.sync.dma_start(
                    xt[:], xqT[k * 128:(k + 1) * 128,
                               c * N_CHUNK:(c + 1) * N_CHUNK])
                tiles.append(xt)
            return tiles

        # ---- bias row (tiny, lands first) ----
        bias_row = consts.tile([1, OUT], mybir.dt.bfloat16)
        nc.scalar.dma_start(bias_row[:], b.rearrange("(p o) -> p o", p=1))
        ones_col = consts.tile([1, 128], mybir.dt.bfloat16)
        nc.vector.memset(ones_col[:], 1.0)

        # ---- startup-critical loads split across BOTH hwdge queues so
        # descriptor issue (~0.6us per DMA instruction) parallelizes:
        # sync queue carries chunk-0 x, scalar queue carries W out-slice 0.
        # The remaining W slices follow on the sync queue; they are needed
        # ~27us/slice later, well after their issue+stream time. ----
        wt_tiles = [consts.tile([128, OUT], mybir.dt.bfloat16, name=f"wt{k}")
                    for k in range(K_TILES)]
        xq0 = load_chunk(0)
        for k in range(K_TILES):
            nc.scalar.dma_start(wt_tiles[k][:, 0:512],
                                wT[k * 128:(k + 1) * 128, 0:512])
        for ob in range(1, O_BANKS):
            for k in range(K_TILES):
                nc.sync.dma_start(
                    wt_tiles[k][:, ob * 512:(ob + 1) * 512],
                    wT[k * 128:(k + 1) * 128, ob * 512:(ob + 1) * 512])

        # ---- bias broadcast via ones-matmul; repeated rounds double as
        # PE warmup so the HAM clock gate opens before the real GEMM ----
        bias_bc = consts.tile([128, OUT], mybir.dt.bfloat16)
        for rep in range(5):
            for ob in range(O_BANKS):
                pt = psum.tile([128, 512], mybir.dt.float32, name="pt",
                               tag=f"acc{ob}")
                nc.tensor.matmul(pt[:], ones_col[:],
                                 bias_row[:, ob * 512:(ob + 1) * 512])
                if rep == 4:
                    nc.scalar.copy(bias_bc[:, ob * 512:(ob + 1) * 512], pt[:])

        # ---- main loop: ob-outer so only one W out-slice gates startup ----
        for c in range(NS // N_CHUNK):
            xqs = xq0 if c == 0 else load_chunk(c)
            n0 = c * N_CHUNK
            for ob in range(O_BANKS):
                o0 = ob * 512
                for rt in range(RT):
                    ps = psum.tile([128, 512], mybir.dt.float32,
                                   name=f"acc{rt % 4}", tag=f"acc{rt % 4}")
                    for k in range(K_TILES):
                        nc.tensor.matmul(
                            ps[:],
                            xqs[k][:, rt * 128:(rt + 1) * 128],
                            wt_tiles[k][:, o0:o0 + 512],
                            start=(k == 0), stop=(k == K_TILES - 1))
                    # scalar engine drains PSUM (bank release flows through
                    # the Scalar queue, immune to Vector-queue forwarder
                    # stalls); DVE then adds bias at 2x bf16 rate
                    og = op.tile([128, 512], mybir.dt.bfloat16, name="og")
                    nc.scalar.copy(og[:], ps[:])
                    ot = op.tile([128, 512], mybir.dt.bfloat16, name="ot")
                    nc.vector.tensor_add(ot[:], og[:], bias_bc[:, o0:o0 + 512])
                    nc.scalar.dma_start(
                        out[n0 + rt * 128:n0 + (rt + 1) * 128, o0:o0 + 512],
                        ot[:])
    nc.finalize()
    return nc


def _quantize_host(x, scale):
    # Bit-identical to reference.quantize_dequant_fp8's quantization step:
    # bf16 divide by bf16 scale, then RNE cast to float8_e4m3fn.
    xb = x.astype(ml_dtypes.bfloat16)
    if scale != 1.0:
        xb = (xb / np.array(scale, dtype=ml_dtypes.bfloat16)).astype(
            ml_dtypes.bfloat16)
    return xb.astype(ml_dtypes.float8_e4m3fn)


def kernel(x, weight, bias, input_scale, _trace=False):
    s = float(np.asarray(input_scale).reshape(-1)[0])
    if "nc" not in _cache:
        _cache["nc"] = build()
    nc = _cache["nc"]

    if s != 1.0:
        wT = np.ascontiguousarray(
            (weight.astype(np.float32) * s).astype(ml_dtypes.bfloat16).T)
    else:
        wT = np.ascontiguousarray(np.asarray(weight).T)
    bias = np.ascontiguousarray(bias)
    xq = _quantize_host(np.asarray(x), s)          # [N, IN] fp8
    in_maps = [
        {"xqT": np.ascontiguousarray(xq[i * NS:(i + 1) * NS].T),
         "wT": wT, "bias": bias}
        for i in range(N_CORES)
    ]
    res = run_bass_kernel_spmd(nc, in_maps, core_ids=list(range(N_CORES)),
                               trace=_trace)
    outs = [res.results[i]["out"] for i in range(N_CORES)]
    full = np.concatenate(outs, axis=0)
    if _trace:
        return full, res
    return full


# revision 11
# speedup vs baseline: 1.0147x; 1.0096x over previous
"""InputScaledQuantLinear on 8 TRN2 NeuronCores.

out = dq(fp8_quant(x / s)) * s @ W^T + bias

Sharding: x rows split 8 ways (data parallel), weight/bias replicated.
Host pre-packs per-shard inputs so every device DMA is contiguous:
  - xqT: e4m3(x/s) transposed to [IN, NS] (the e4m3 quantization is
    bit-identical to the reference, so that error cancels)
  - wT:  (s * W)^T as [IN, OUT] bf16
Device is then a pure dense bf16-rate GEMM: stationary = xq^T row-tile,
moving = wT out-slice, accumulate K=2048 in PSUM, bias-add on DVE.
"""

import numpy as np
import ml_dtypes
from contextlib import ExitStack

import concourse.bass as bass
import concourse.mybir as mybir
import concourse.tile as tile
from concourse import bacc
from concourse.bass_utils import run_bass_kernel_spmd

N_CORES = 8
N, IN, OUT = 32768, 2048, 2048
NS = N // N_CORES          # 4096 rows per core
N_CHUNK = 1024             # token rows per outer iteration
K_TILES = IN // 128        # 16
O_BANKS = OUT // 512       # 4
RT = N_CHUNK // 128        # 8 row-tiles per chunk

_cache = {}


def build():
    nc = bacc.Bacc(trn_type="TRN2")
    xqT = nc.dram_tensor("xqT", [IN, NS], mybir.dt.float8e4, kind="ExternalInput")
    wT = nc.dram_tensor("wT", [IN, OUT], mybir.dt.bfloat16, kind="ExternalInput")
    b = nc.dram_tensor("bias", [OUT], mybir.dt.bfloat16, kind="ExternalInput")
    out = nc.dram_tensor("out", [NS, OUT], mybir.dt.bfloat16, kind="ExternalOutput")

    with tile.TileContext(nc) as tc, ExitStack() as ctx:
        consts = ctx.enter_context(tc.tile_pool(name="consts", bufs=1))
        xp = ctx.enter_context(tc.tile_pool(name="xp", bufs=2))
        op = ctx.enter_context(tc.tile_pool(name="op", bufs=8))
        psum = ctx.enter_context(tc.tile_pool(name="psum", bufs=2, space="PSUM"))

        # ---- x chunk loads: 16 contiguous [128, N_CHUNK] fp8 tiles ----
        def load_chunk(c):
            tiles = []
            for k in range(K_TILES):
                xt = xp.tile([128, N_CHUNK], mybir.dt.float8e4, name=f"xq{k}",
                             tag=f"xq{k}")
                nc.sync.dma_start(
                    xt[:], xqT[k * 128:(k + 1) * 128,
                               c * N_CHUNK:(c + 1) * N_CHUNK])
                tiles.append(xt)
            return tiles

        # ---- PE warmup: 4 dependency-free matmuls on memset tiles so the
        # HAM clock gate opens before the real GEMM ----
        ones_col = consts.tile([1, 128], mybir.dt.bfloat16)
        nc.vector.memset(ones_col[:], 1.0)
        ones_row = consts.tile([1, 512], mybir.dt.bfloat16)
        nc.vector.memset(ones_row[:], 1.0)
        for ob in range(O_BANKS):
            pt = psum.tile([128, 512], mybir.dt.float32, name="pt",
                           tag=f"acc{ob}")
            nc.tensor.matmul(pt[:], ones_col[:], ones_row[:])

        # ---- startup-critical loads split across BOTH hwdge queues so
        # descriptor issue (~0.6us per DMA instruction) parallelizes:
        # sync queue carries chunk-0 x, scalar queue carries the bias and
        # W out-slice 0. The remaining W slices follow on the sync queue;
        # they are needed ~27us/slice later, well after issue+stream. ----
        bias_bc = consts.tile([128, OUT], mybir.dt.bfloat16)
        bias_src = b.rearrange("(p o) -> p o", p=1).partition_broadcast(128)
        wt_tiles = [consts.tile([128, OUT], mybir.dt.bfloat16, name=f"wt{k}")
                    for k in range(K_TILES)]
        nc.scalar.dma_start(bias_bc[:, 0:512], bias_src[:, 0, 0:512])
        xq0 = load_chunk(0)
        for k in range(K_TILES):
            nc.scalar.dma_start(wt_tiles[k][:, 0:512],
                                wT[k * 128:(k + 1) * 128, 0:512])
        for ob in range(1, O_BANKS):
            nc.scalar.dma_start(bias_bc[:, ob * 512:(ob + 1) * 512],
                                bias_src[:, 0, ob * 512:(ob + 1) * 512])
        for ob in range(1, O_BANKS):
            for k in range(K_TILES):
                nc.sync.dma_start(
                    wt_tiles[k][:, ob * 512:(ob + 1) * 512],
                    wT[k * 128:(k + 1) * 128, ob * 512:(ob + 1) * 512])

        # ---- main loop: ob-outer so only one W out-slice gates startup ----
        for c in range(NS // N_CHUNK):
            xqs = xq0 if c == 0 else load_chunk(c)
            n0 = c * N_CHUNK
            for ob in range(O_BANKS):
                o0 = ob * 512
                for rt in range(RT):
                    ps = psum.tile([128, 512], mybir.dt.float32,
                                   name=f"acc{rt % 4}", tag=f"acc{rt % 4}")
                    for k in range(K_TILES):
                        nc.tensor.matmul(
                            ps[:],
                            xqs[k][:, rt * 128:(rt + 1) * 128],
                            wt_tiles[k][:, o0:o0 + 512],
                            start=(k == 0), stop=(k == K_TILES - 1))
                    # scalar engine drains PSUM (bank release flows through
                    # the Scalar queue, immune to Vector-queue forwarder
                    # stalls); DVE then adds bias at 2x bf16 rate
                    og = op.tile([128, 512], mybir.dt.bfloat16, name="og")
                    nc.scalar.copy(og[:], ps[:])
                    ot = op.tile([128, 512], mybir.dt.bfloat16, name="ot")
                    nc.vector.tensor_add(ot[:], og[:], bias_bc[:, o0:o0 + 512])
                    last = (c == NS // N_CHUNK - 1 and ob == O_BANKS - 1
                            and rt == RT - 1)
                    if last:
                        # final tile: 4-way split so the tail store streams
                        # on 4 DMA engines instead of 1
                        for q in range(4):
                            nc.scalar.dma_start(
                                out[n0 + rt * 128:n0 + (rt + 1) * 128,
                                    o0 + q * 128:o0 + (q + 1) * 128],
                                ot[:, q * 128:(q + 1) * 128])
                    else:
                        nc.scalar.dma_start(
                            out[n0 + rt * 128:n0 + (rt + 1) * 128,
                                o0:o0 + 512],
                            ot[:])
    nc.finalize()
    return nc


def _quantize_host(x, scale):
    # Bit-identical to reference.quantize_dequant_fp8's quantization step:
    # bf16 divide by bf16 scale, then RNE cast to float8_e4m3fn.
    xb = x.astype(ml_dtypes.bfloat16)
    if scale != 1.0:
        xb = (xb / np.array(scale, dtype=ml_dtypes.bfloat16)).astype(
            ml_dtypes.bfloat16)
    return xb.astype(ml_dtypes.float8_e4m3fn)


def kernel(x, weight, bias, input_scale, _trace=False):
    s = float(np.asarray(input_scale).reshape(-1)[0])
    if "nc" not in _cache:
        _cache["nc"] = build()
    nc = _cache["nc"]

    if s != 1.0:
        wT = np.ascontiguousarray(
            (weight.astype(np.float32) * s).astype(ml_dtypes.bfloat16).T)
    else:
        wT = np.ascontiguousarray(np.asarray(weight).T)
    bias = np.ascontiguousarray(bias)
    xq = _quantize_host(np.asarray(x), s)          # [N, IN] fp8
    in_maps = [
        {"xqT": np.ascontiguousarray(xq[i * NS:(i + 1) * NS].T),
         "wT": wT, "bias": bias}
        for i in range(N_CORES)
    ]
    res = run_bass_kernel_spmd(nc, in_maps, core_ids=list(range(N_CORES)),
                               trace=_trace)
    outs = [res.results[i]["out"] for i in range(N_CORES)]
    full = np.concatenate(outs, axis=0)
    if _trace:
        return full, res
    return full


# revision 12
# speedup vs baseline: 1.0337x; 1.0187x over previous
"""InputScaledQuantLinear on 8 TRN2 NeuronCores.

out = dq(fp8_quant(x / s)) * s @ W^T + bias

Sharding: x rows split 8 ways (data parallel), weight/bias replicated.
Host pre-packs per-shard inputs so every device DMA is contiguous:
  - xqT: e4m3(x/s) transposed to [IN, NS] (the e4m3 quantization is
    bit-identical to the reference, so that error cancels)
  - wT:  (s * W)^T as [IN, OUT] bf16
Device is then a pure dense bf16-rate GEMM: stationary = xq^T row-tile,
moving = wT out-slice, accumulate K=2048 in PSUM, bias-add on DVE.
"""

import numpy as np
import ml_dtypes
from contextlib import ExitStack

import concourse.bass as bass
import concourse.mybir as mybir
import concourse.tile as tile
from concourse import bacc
from concourse.bass_utils import run_bass_kernel_spmd

N_CORES = 8
N, IN, OUT = 32768, 2048, 2048
NS = N // N_CORES          # 4096 rows per core
N_CHUNK = 1024             # token rows per outer iteration
K_TILES = IN // 128        # 16
O_BANKS = OUT // 512       # 4
RT = N_CHUNK // 128        # 8 row-tiles per chunk

_cache = {}


def build():
    nc = bacc.Bacc(trn_type="TRN2")
    xqT = nc.dram_tensor("xqT", [IN, NS], mybir.dt.float8e4, kind="ExternalInput")
    wT = nc.dram_tensor("wT", [IN, OUT], mybir.dt.bfloat16, kind="ExternalInput")
    b = nc.dram_tensor("bias", [OUT], mybir.dt.bfloat16, kind="ExternalInput")
    out = nc.dram_tensor("out", [NS, OUT], mybir.dt.bfloat16, kind="ExternalOutput")

    with tile.TileContext(nc) as tc, ExitStack() as ctx:
        consts = ctx.enter_context(tc.tile_pool(name="consts", bufs=1))
        xp = ctx.enter_context(tc.tile_pool(name="xp", bufs=2))
        op = ctx.enter_context(tc.tile_pool(name="op", bufs=8))
        psum = ctx.enter_context(tc.tile_pool(name="psum", bufs=2, space="PSUM"))

        # ---- x chunk loads: 16 contiguous [128, N_CHUNK] fp8 tiles ----
        def load_chunk(c):
            tiles = []
            for k in range(K_TILES):
                xt = xp.tile([128, N_CHUNK], mybir.dt.float8e4, name=f"xq{k}",
                             tag=f"xq{k}")
                nc.sync.dma_start(
                    xt[:], xqT[k * 128:(k + 1) * 128,
                               c * N_CHUNK:(c + 1) * N_CHUNK])
                tiles.append(xt)
            return tiles

        # ---- PE warmup: 4 dependency-free matmuls on memset tiles so the
        # HAM clock gate opens before the real GEMM ----
        ones_col = consts.tile([1, 128], mybir.dt.bfloat16)
        nc.vector.memset(ones_col[:], 1.0)
        ones_row = consts.tile([1, 512], mybir.dt.bfloat16)
        nc.vector.memset(ones_row[:], 1.0)
        for ob in range(O_BANKS):
            pt = psum.tile([128, 512], mybir.dt.float32, name="pt",
                           tag=f"acc{ob}")
            nc.tensor.matmul(pt[:], ones_col[:], ones_row[:])

        # ---- startup-critical loads split across BOTH hwdge queues so
        # descriptor issue (~0.6us per DMA instruction) parallelizes:
        # sync queue carries chunk-0 x, scalar queue carries the bias and
        # W out-slice 0. The remaining W slices follow on the sync queue;
        # they are needed ~27us/slice later, well after issue+stream. ----
        bias_bc = consts.tile([128, OUT], mybir.dt.bfloat16)
        bias_src = b.rearrange("(p o) -> p o", p=1).partition_broadcast(128)
        wt_tiles = [consts.tile([128, OUT], mybir.dt.bfloat16, name=f"wt{k}")
                    for k in range(K_TILES)]
        nc.scalar.dma_start(bias_bc[:, 0:512], bias_src[:, 0, 0:512])
        xq0 = load_chunk(0)
        for k in range(K_TILES):
            nc.scalar.dma_start(wt_tiles[k][:, 0:512],
                                wT[k * 128:(k + 1) * 128, 0:512])
        for ob in range(1, O_BANKS):
            nc.scalar.dma_start(bias_bc[:, ob * 512:(ob + 1) * 512],
                                bias_src[:, 0, ob * 512:(ob + 1) * 512])
        for ob in range(1, O_BANKS):
            for k in range(K_TILES):
                nc.sync.dma_start(
                    wt_tiles[k][:, ob * 512:(ob + 1) * 512],
                    wT[k * 128:(k + 1) * 128, ob * 512:(ob + 1) * 512])

        # ---- main loop: ob-outer so only one W out-slice gates startup ----
        for c in range(NS // N_CHUNK):
            xqs = xq0 if c == 0 else load_chunk(c)
            n0 = c * N_CHUNK
            for ob in range(O_BANKS):
                o0 = ob * 512
                for rt in range(RT):
                    ps = psum.tile([128, 512], mybir.dt.float32,
                                   name=f"acc{rt % 4}", tag=f"acc{rt % 4}")
                    for k in range(K_TILES):
                        nc.tensor.matmul(
                            ps[:],
                            xqs[k][:, rt * 128:(rt + 1) * 128],
                            wt_tiles[k][:, o0:o0 + 512],
                            start=(k == 0), stop=(k == K_TILES - 1))
                    # scalar engine drains PSUM (bank release flows through
                    # the Scalar queue, immune to Vector-queue forwarder
                    # stalls); DVE then adds bias at 2x bf16 rate
                    og = op.tile([128, 512], mybir.dt.bfloat16, name="og")
                    nc.scalar.copy(og[:], ps[:])
                    ot = op.tile([128, 512], mybir.dt.bfloat16, name="ot")
                    nc.vector.tensor_add(ot[:], og[:], bias_bc[:, o0:o0 + 512])
                    last = (c == NS // N_CHUNK - 1 and ob == O_BANKS - 1
                            and rt == RT - 1)
                    if last:
                        # final tile: 4-way split so the tail store streams
                        # on 4 DMA engines instead of 1
                        for q in range(4):
                            nc.scalar.dma_start(
                                out[n0 + rt * 128:n0 + (rt + 1) * 128,
                                    o0 + q * 128:o0 + (q + 1) * 128],
                                ot[:, q * 128:(q + 1) * 128])
                    else:
                        nc.scalar.dma_start(
                            out[n0 + rt * 128:n0 + (rt + 1) * 128,
                                o0:o0 + 512],
                            ot[:])
    nc.finalize()
    return nc


def _quantize_host(x, scale):
    # Bit-identical to reference.quantize_dequant_fp8's quantization step:
    # bf16 divide by bf16 scale, then RNE cast to float8_e4m3fn.
    xb = x.astype(ml_dtypes.bfloat16)
    if scale != 1.0:
        xb = (xb / np.array(scale, dtype=ml_dtypes.bfloat16)).astype(
            ml_dtypes.bfloat16)
    return xb.astype(ml_dtypes.float8_e4m3fn)


def kernel(x, weight, bias, input_scale, _trace=False):
    s = float(np.asarray(input_scale).reshape(-1)[0])
    if "nc" not in _cache:
        _cache["nc"] = build()
    nc = _cache["nc"]

    weight = np.asarray(weight)
    if s != 1.0:
        wT = np.ascontiguousarray(
            (weight.astype(np.float32) * s).astype(ml_dtypes.bfloat16).T)
    else:
        wT = np.ascontiguousarray(weight.T)
    bias = np.ascontiguousarray(bias)
    xq = _quantize_host(np.asarray(x), s)          # [N, IN] fp8
    in_maps = [
        {"xqT": np.ascontiguousarray(xq[i * NS:(i + 1) * NS].T),
         "wT": wT, "bias": bias}
        for i in range(N_CORES)
    ]
    res = run_bass_kernel_spmd(nc, in_maps, core_ids=list(range(N_CORES)),
                               trace=_trace)
    outs = [res.results[i]["out"] for i in range(N_CORES)]
    full = np.concatenate(outs, axis=0)
    if _trace:
        return full, res
    return full
